# revision 7
# baseline (speedup 1.0000x reference)
"""Trainium2 Bass kernel: embedding gather + segment mean (8-core SPMD).

Strategy (v5):
  - Split the 25000 segments evenly across 8 cores (3125 each); each core
    handles the tokens of its own segments (host-computed from segment_ids).
  - Per core, segments are grouped into 25 windows of 125 (3125 = 25*125).
    Each window's gather uses gpsimd.dma_gather (InstDMAGatherAnt): gathered
    row i lands at SBUF partition i%128, column i//128, so with list position
    i = j*128 + p the tile is exactly [seg p, word j, feature] — no
    reassociation needed. num_idxs is capped at 1024 per op (the SWDGE
    descriptor ring holds ~65-80 descs/engine; 1280 wedges the device), so a
    window is gathered in ceil(maxlen/8) j-block ops, rotated across the 4
    SWDGE queues (queue q runs on Q7 core pair 2q/2q+1 -> parallel descgen).
  - dma_gather takes int16 indices, so the host re-lays-out the embedding
    table per core: one block per window holding that window's unique rows
    (<= 125*maxlen < 32767), bf16, padded to 384 cols (768 B rows, a multiple
    of the 256 B descriptor-stride granule). Local indices fit int16. Device
    still moves every token's 768 B row from HBM (same traffic as a plain
    gather); the host only permutes/duplicates table rows.
  - A vector-engine tensor_reduce over the word axis gives f32 segment sums;
    multiplying by host-provided 1/count gives means. Host reassembles the
    [25000, 300] output from the per-core [25 windows, 125, 300] outputs.
"""
import sys
sys.path.insert(0, "/opt/trn_rl_repo")

import numpy as np
import ml_dtypes

VOCAB = 517015
D = 300
DPAD = 384          # bf16 row padded to 768 B (256 B multiple)
S_TOTAL = 25_000
N_CORES = 8
S_CORE = S_TOTAL // N_CORES      # 3125
WIN = 125
N_WIN = S_CORE // WIN            # 25

_cache = {}


class _Runner:
    """Compile a Bass module once and run it repeatedly on 8 cores via PJRT."""

    def __init__(self, nc, n_cores):
        import jax
        from jax.sharding import Mesh, PartitionSpec, NamedSharding
        from jax.experimental.shard_map import shard_map
        from concourse import bass2jax, mybir

        self.jax = jax
        self.n_cores = n_cores
        bass2jax.install_neuronx_cc_hook()
        partition_name = (nc.partition_id_tensor.name
                          if nc.partition_id_tensor else None)
        in_names, out_names, out_avals, zero_outs = [], [], [], []
        for alloc in nc.m.functions[0].allocations:
            if not isinstance(alloc, mybir.MemoryLocationSet):
                continue
            name = alloc.memorylocations[0].name
            if alloc.kind == "ExternalInput":
                if name != partition_name:
                    in_names.append(name)
            elif alloc.kind == "ExternalOutput":
                shape = tuple(alloc.tensor_shape)
                dtype = mybir.dt.np(alloc.dtype)
                out_names.append(name)
                out_avals.append(jax.core.ShapedArray(shape, dtype))
                zero_outs.append(np.zeros(shape, dtype))
        n_params = len(in_names)
        all_in = list(in_names) + list(out_names)
        if partition_name is not None:
            all_in.append(partition_name)

        def _body(*args):
            operands = list(args)
            if partition_name is not None:
                operands.append(bass2jax.partition_id_tensor())
            return tuple(bass2jax._bass_exec_p.bind(
                *operands,
                out_avals=tuple(out_avals),
                in_names=tuple(all_in),
                out_names=tuple(out_names),
                lowering_input_output_aliases=(),
                sim_require_finite=True,
                sim_require_nnan=True,
                nc=nc,
            ))

        devices = jax.devices()[:n_cores]
        mesh = Mesh(np.asarray(devices), ("core",))
        n_all = n_params + len(out_names)
        self.fn = jax.jit(
            shard_map(_body, mesh=mesh,
                      in_specs=(PartitionSpec("core"),) * n_all,
                      out_specs=(PartitionSpec("core"),) * len(out_names),
                      check_rep=False),
            keep_unused=True)
        self.sharding = NamedSharding(mesh, PartitionSpec("core"))
        self.in_names = in_names
        self.out_names = out_names
        self.out_avals = out_avals
        self.zero_outs = zero_outs

    def device_args(self, in_maps):
        args = []
        for name in self.in_names:
            cat = np.concatenate([np.asarray(m[name]) for m in in_maps], axis=0)
            args.append(self.jax.device_put(cat, self.sharding))
        for z in self.zero_outs:
            cat = np.zeros((self.n_cores * z.shape[0], *z.shape[1:]), z.dtype)
            args.append(self.jax.device_put(cat, self.sharding))
        return args

    def run_args(self, args):
        outs = self.jax.block_until_ready(self.fn(*args))
        return [
            {name: np.asarray(outs[i]).reshape(
                self.n_cores, *self.out_avals[i].shape)[c]
             for i, name in enumerate(self.out_names)}
            for c in range(self.n_cores)
        ]

    def run(self, in_maps):
        return self.run_args(self.device_args(in_maps))


def _block_rows(maxlen):
    return 128 * maxlen + 16     # unique rows + zero row(s); NI pads hit row NI


def _build(maxlen, iters=1):
    import concourse.bacc as bacc
    import concourse.tile as tile
    from concourse import mybir
    from concourse.library_config import mlp

    NI = 128 * maxlen            # num_idxs per window (row i -> [i%128, i//128])
    NC16 = NI // 16              # idx columns per window
    BLOCK = _block_rows(maxlen)

    nc = bacc.Bacc("TRN2", target_bir_lowering=False, debug=False,
                   num_devices=N_CORES, num_swdge_queues=4)
    table = nc.dram_tensor("table", [N_WIN * BLOCK, DPAD], mybir.dt.bfloat16,
                           kind="ExternalInput")
    idx = nc.dram_tensor("idx", [128, N_WIN * NC16], mybir.dt.int16,
                         kind="ExternalInput")
    invc = nc.dram_tensor("invc", [128, N_WIN], mybir.dt.float32,
                          kind="ExternalInput")
    out = nc.dram_tensor("out", [N_WIN, WIN, D], mybir.dt.float32,
                         kind="ExternalOutput")

    with tile.TileContext(nc) as tc:
        with tc.tile_pool(name="const", bufs=1) as cpool, \
             tc.tile_pool(name="gather", bufs=3) as gpool, \
             tc.tile_pool(name="res", bufs=3) as rpool:
            nc.gpsimd.load_library(mlp)
            idx_t = cpool.tile([128, N_WIN * NC16], mybir.dt.int16)
            nc.sync.dma_start(out=idx_t[:], in_=idx[:])
            invc_t = cpool.tile([128, N_WIN], mybir.dt.float32)
            nc.sync.dma_start(out=invc_t[:], in_=invc[:])

            opi = 0
            for it in range(iters):
              for w in range(N_WIN):
                g = gpool.tile([128, maxlen, DPAD], mybir.dt.bfloat16, tag="g")
                for j0 in range(0, maxlen, 8):
                    j1 = min(j0 + 8, maxlen)
                    nb = (j1 - j0) * 128
                    nc.gpsimd.dma_gather(
                        out_ap=g[:, j0:j1, :],
                        in_ap=table[w * BLOCK:(w + 1) * BLOCK],
                        idxs_ap=idx_t[:, w * NC16 + j0 * 8:
                                      w * NC16 + j1 * 8],
                        num_idxs=nb,
                        num_idxs_reg=nb,
                        elem_size=DPAD,
                        queue_num=opi % 4,
                    )
                    opi += 1
                s = rpool.tile([128, D], mybir.dt.float32, tag="s")
                gv = g[:WIN].rearrange("p j d -> p d j")[:, :D, :]
                nc.vector.tensor_reduce(out=s[:WIN], in_=gv,
                                        axis=mybir.AxisListType.X,
                                        op=mybir.AluOpType.add)
                m = rpool.tile([128, D], mybir.dt.float32, tag="m")
                nc.vector.tensor_tensor(
                    out=m[:WIN], in0=s[:WIN],
                    in1=invc_t[:WIN, w:w + 1].to_broadcast([WIN, D]),
                    op=mybir.AluOpType.mult)
                nc.sync.dma_start(out=out[w], in_=m[:WIN])
    nc.compile()
    return nc


def get_runner(maxlen, iters=1):
    key = ("v5", maxlen, iters)
    if key not in _cache:
        _cache[key] = _Runner(_build(maxlen, iters), N_CORES)
    return _cache[key]


def prepare_inputs(word_emb, word_ids, segment_ids, num_segments):
    """Host-side sharding/metadata prep. Returns (maxlen, in_maps)."""
    word_emb = np.asarray(word_emb, dtype=np.float32)
    word_ids = np.asarray(word_ids).astype(np.int64)
    segment_ids = np.asarray(segment_ids).astype(np.int64)
    S = int(num_segments)
    T = word_ids.shape[0]
    assert S == S_TOTAL and word_emb.shape == (VOCAB, D)

    counts = np.bincount(segment_ids, minlength=S).astype(np.int64)
    maxlen = int(counts.max())
    assert maxlen <= 64, "segment too long for single-pass kernel"
    NI = 128 * maxlen
    NC16 = NI // 16
    BLOCK = _block_rows(maxlen)
    seg_starts = np.zeros(S + 1, dtype=np.int64)
    np.cumsum(counts, out=seg_starts[1:])
    with np.errstate(divide="ignore"):
        inv_counts = (1.0 / counts.astype(np.float32)).astype(np.float32)

    # per-token coordinates
    t = np.arange(T, dtype=np.int64)
    seg = segment_ids
    j = t - seg_starts[seg]                  # position within segment
    c_arr = seg // S_CORE
    loc = seg % S_CORE
    w_arr = loc // WIN
    p_arr = loc % WIN
    gw = c_arr * N_WIN + w_arr               # global window id

    # per-window unique word ids -> local int16 codes + compact table blocks
    order = np.lexsort((word_ids, gw))
    sw, swid = gw[order], word_ids[order]
    new_blk = np.r_[True, sw[1:] != sw[:-1]]
    new_val = new_blk | np.r_[True, swid[1:] != swid[:-1]]
    uniq_cum = np.cumsum(new_val) - 1                    # global unique counter
    blk_of_sorted = np.cumsum(new_blk) - 1               # 0..(8*N_WIN-1)
    base_per_blk = uniq_cum[np.flatnonzero(new_blk)]
    local_sorted = uniq_cum - base_per_blk[blk_of_sorted]
    assert local_sorted.max() < NI
    local = np.empty(T, dtype=np.int64)
    local[order] = local_sorted

    # compact table: [8, N_WIN*BLOCK, DPAD] bf16
    emb_bf = word_emb.astype(ml_dtypes.bfloat16)
    big_table = np.zeros((N_CORES, N_WIN * BLOCK, DPAD), dtype=ml_dtypes.bfloat16)
    u_mask = new_val
    u_gw = sw[u_mask]
    u_row = (u_gw % N_WIN) * BLOCK + local_sorted[u_mask]
    big_table[u_gw // N_WIN, u_row, :D] = emb_bf[swid[u_mask]]

    # int16 index lists: position i = j*128 + p; wrapped [128, NC16] per window
    idx_lists = np.full((N_CORES, N_WIN, NI), NI, dtype=np.int16)  # NI = zero row
    idx_lists[c_arr, w_arr, j * 128 + p_arr] = local.astype(np.int16)
    # wrap: entry i -> [i%16, i//16], replicated across the 8 partition groups
    wrapped = idx_lists.reshape(N_CORES, N_WIN, NC16, 16)          # [c,w,col,part%16]
    big_idx = np.empty((N_CORES, 128, N_WIN * NC16), dtype=np.int16)
    big_idx[:] = wrapped.transpose(0, 3, 1, 2).reshape(
        N_CORES, 1, 16, N_WIN * NC16).repeat(8, axis=1).reshape(
        N_CORES, 128, N_WIN * NC16)

    big_invc = np.zeros((N_CORES, 128, N_WIN), dtype=np.float32)
    s_all = np.arange(S, dtype=np.int64)
    big_invc[s_all // S_CORE, (s_all % S_CORE) % WIN,
             (s_all % S_CORE) // WIN] = inv_counts

    in_maps = [{"table": big_table[c], "idx": big_idx[c], "invc": big_invc[c]}
               for c in range(N_CORES)]
    return maxlen, in_maps


def assemble_output(results):
    out = np.empty((S_TOTAL, D), dtype=np.float32)
    for c in range(N_CORES):
        o = results[c]["out"].reshape(S_CORE, D)
        out[c * S_CORE:(c + 1) * S_CORE] = o
    return out


def kernel(word_emb, word_ids, segment_ids, num_segments):
    maxlen, in_maps = prepare_inputs(word_emb, word_ids, segment_ids,
                                     num_segments)
    runner = get_runner(maxlen)
    results = runner.run(in_maps)
    return assemble_output(results)
